# revision 8
# baseline (speedup 1.0000x reference)
"""Multi-headed self-attention Trainium2 kernel (8 NeuronCores), v2.

Problem: B=4, S=2048, D=768, H=12 heads of DH=64; fp32 inputs.
Sharding: core c handles batch b = c//2 and head group g = c%2 (6 heads).

v2 reorganizes the v1 kernel around the ScalarE (ACT) critical path:
the 25.2M-element exp is the steady-state bottleneck (1 elem/lane/cyc
@1.2GHz => 164us floor + 293ns/instr overhead).

  - Score tiles are pair-aligned: one [128,1024] fp32 PSUM tile per key
    block = {head A | head B}, written by a row-tiled concurrent MM pair
    (tile_position (0,0)/(64,0), different PSUM banks). Both halves are
    freed by ONE ACTIVATE, so the next pair's deps clear atomically.
  - Emission is software-pipelined over a continuous kb-stream g=0..191
    (12 stripe-passes x 16 key blocks): per period emit ACT(g),
    pair(g+2), attnV(g-1), then deadline-driven projection chunks.
    No stripe-boundary ACT bubbles.
  - Prologue: 9 coarse DMAs in dependency order (biases+mask, wq,
    xT stripe 0, wk, wv, xT stripes 1-3); PE warmup dummies unthrottle
    the HAM clock gate before the first projection.
  - Epilogue (no DMA round trips; the strict-FIFO DVE/Sync queues must
    never head-of-line block on a multi-us dependency): the acc copy
    writes fp16; a constant selector matmul broadcasts the denominator
    row onto partitions 0-63 in PSUM; reciprocal_approx_fast + mul +
    bias add on DVE; stages are deferred 1-2 periods so each op's
    inputs exist before it reaches its queue head.

Algebra (unchanged from v1): bv is added after normalization (factors
out of the softmax average); the padding mask is folded into V
multiplicatively with a mask-valued denominator column, so attn@V_aug
also produces the softmax denominator (row 64).
"""

import numpy as np

B, S, D, H = 4, 2048, 768, 12
DH = 64          # head dim
HPC = 6          # heads per core
DHC = HPC * DH   # 384 = per-core slice of D
N_CORES = 8
P = 128
KC = D // P      # 6 contraction chunks
NSK = S // P     # 16 key blocks
NQS = S // 512   # 4 query stripes per m-block
NPASS = 3 * NQS  # 12 stripe-passes
NG = NPASS * NSK # 192 kb-stream periods

_CACHED = None


def _build_module():
    import concourse.bacc as bacc
    import concourse.tile as tile
    from concourse import mybir

    f32 = mybir.dt.float32
    f16 = mybir.dt.float16
    i32 = mybir.dt.int32
    EXP = mybir.ActivationFunctionType.Exp

    nc = bacc.Bacc(trn_type="TRN2")

    xT = nc.dram_tensor("xT", [D, S], f16, kind="ExternalInput")
    wq = nc.dram_tensor("wq", [D, DHC], f16, kind="ExternalInput")
    wk = nc.dram_tensor("wk", [D, DHC], f16, kind="ExternalInput")
    wv = nc.dram_tensor("wv", [D, DHC], f16, kind="ExternalInput")
    # bias_pack [128, 9]: cols 0-2 = bq (col=mb), 3-5 = bk, 6-8 = bv
    biasp = nc.dram_tensor("biasp", [P, 9], f32, kind="ExternalInput")
    maskc = nc.dram_tensor("maskc", [P, NSK], i32, kind="ExternalInput")
    out = nc.dram_tensor("out", [DHC, S], f16, kind="ExternalOutput")

    xT4 = xT.rearrange("(c p) s -> p c s", p=P)
    wq3 = wq.rearrange("(c p) n -> p c n", p=P)
    wk3 = wk.rearrange("(c p) n -> p c n", p=P)
    wv3 = wv.rearrange("(c p) n -> p c n", p=P)

    with tile.TileContext(nc) as tc:
        sb = tc.alloc_tile_pool(name="sb", bufs=1)
        attnp = tc.alloc_tile_pool(name="attnp", bufs=10)
        ep = tc.alloc_tile_pool(name="ep", bufs=3)
        # PSUM: 2 score tiles (4 banks) + 2 proj slots (2) + 2 acc (2) = 8
        ps = tc.alloc_tile_pool(name="ps", bufs=2, space="PSUM")
        pj = tc.alloc_tile_pool(name="pj", bufs=2, space="PSUM")
        accp = tc.alloc_tile_pool(name="accp", bufs=2, space="PSUM")

        # ---- PE warmup: dummy matmuls on a scratch tile unthrottle HAM
        warm = sb.tile([P, 512], f16)
        nc.gpsimd.memset(warm, 0)
        # selector for the denominator broadcast: sel.T @ o_raw replicates
        # row 64 (the aug-column denominator) onto partitions 0-63
        sel_sb = sb.tile([P, 64], f16)
        nc.gpsimd.memset(sel_sb, 0)
        nc.gpsimd.memset(sel_sb[64:65, :], 1.0)
        for _ in range(12):
            wps = pj.tile([P, 512], f32, tag="proj", name="warm")
            nc.tensor.matmul(wps, warm[:, 0:128], warm, start=True, stop=True)

        # ---- inputs: coarse DMAs in dependency order ----
        biasp_sb = sb.tile([P, 9], f32)
        nc.sync.dma_start(biasp_sb, biasp[:, :])
        mask_i = sb.tile([P, NSK], i32)
        nc.sync.dma_start(mask_i, maskc[:, :])

        wq_sb = sb.tile([P, KC, DHC], f16)
        nc.sync.dma_start(wq_sb, wq3)

        xT_sb = sb.tile([P, KC, S], f16)
        nc.sync.dma_start(xT_sb[:, :, 0:512], xT4[:, :, 0:512])

        wk_sb = sb.tile([P, KC, DHC], f16)
        nc.sync.dma_start(wk_sb, wk3)

        wv_sb = sb.tile([P, KC, DHC], f16)
        nc.sync.dma_start(wv_sb, wv3)
        for st in range(1, 4):
            nc.sync.dma_start(xT_sb[:, :, st * 512:(st + 1) * 512],
                              xT4[:, :, st * 512:(st + 1) * 512])

        bq_sb = biasp_sb[:, 0:3]
        bk_sb = biasp_sb[:, 3:6]
        bv_sb = biasp_sb[:, 6:9]
        mask_f = sb.tile([P, NSK], f32)
        nc.vector.tensor_copy(mask_f, mask_i)

        # ---- persistent activations ----
        QT_sb = sb.tile([P, 3, S], f16)
        KT_sb = sb.tile([P, 3, S], f16)
        V_sb = sb.tile([P, NSK, HPC * 65], f16)
        V_sb4 = V_sb.rearrange("p n (h e) -> p n h e", e=65)

        def emit_qk_proj_chunk(dst, w_sb, b_sb, mb, ch):
            """One [128, 512] chunk of Q^T or K^T (heads 2mb, 2mb+1)."""
            pps = pj.tile([P, 512], f32, tag="proj", name="pps")
            col = ch * 512
            for c in range(KC):
                nc.tensor.matmul(
                    pps,
                    w_sb[:, c, mb * P:(mb + 1) * P],
                    xT_sb[:, c, col:col + 512],
                    start=(c == 0), stop=(c == KC - 1),
                )
            nc.vector.tensor_scalar(
                dst[:, mb, col:col + 512], pps,
                b_sb[:, mb:mb + 1], None, mybir.AluOpType.add,
            )

        def emit_k_proj_half(mb, ch, half):
            """256 keys of K^T: half a chunk, so a K projection never
            blocks the score-fill path for more than ~1us."""
            pps = pj.tile([P, 256], f32, tag="proj", name="kpp")
            col = ch * 512 + half * 256
            for c in range(KC):
                nc.tensor.matmul(
                    pps,
                    wk_sb[:, c, mb * P:(mb + 1) * P],
                    xT_sb[:, c, col:col + 256],
                    start=(c == 0), stop=(c == KC - 1),
                )
            nc.vector.tensor_scalar(
                KT_sb[:, mb, col:col + 256], pps,
                bk_sb[:, mb:mb + 1], None, mybir.AluOpType.add,
            )

        def emit_v_proj_chunk(sk):
            vps = pj.tile([P, DHC], f32, tag="proj", name="vps")
            for c in range(KC):
                nc.tensor.matmul(
                    vps,
                    xT_sb[:, c, sk * P:(sk + 1) * P],
                    wv_sb[:, c, :],
                    start=(c == 0), stop=(c == KC - 1),
                )
            # evac with the multiplicative mask; fp32 -> fp16
            nc.vector.tensor_scalar(
                V_sb4[:, sk, :, 0:64],
                vps.rearrange("p (h e) -> p h e", e=64),
                mask_f[:, sk:sk + 1], None, mybir.AluOpType.mult,
            )
            nc.vector.tensor_copy(
                V_sb4[:, sk, :, 64],
                mask_f[:, sk:sk + 1].to_broadcast([P, HPC]),
            )

        # deferred projection chunks with deadlines in kb-stream units
        deferred = []
        for sk in range(NSK):
            deferred.append((sk + 2, "v", sk))
        for mb in range(3):
            for ch in range(NQS):
                if not (mb == 0 and ch == 0):
                    deferred.append(((mb * NQS + ch) * NSK - 2, "q", mb, ch))
                    deferred.append((mb * 4 * NSK + ch * 4 - 2, "k", mb, ch, 0))
                    deferred.append((mb * 4 * NSK + ch * 4, "k", mb, ch, 1))
        deferred.sort(key=lambda t: t[0])

        def emit_deferred_due(g):
            while deferred and deferred[0][0] <= g + 2:
                item = deferred.pop(0)
                if item[1] == "v":
                    emit_v_proj_chunk(item[2])
                elif item[1] == "q":
                    emit_qk_proj_chunk(QT_sb, wq_sb, bq_sb, item[2], item[3])
                else:
                    emit_k_proj_half(item[2], item[3], item[4])

        # prologue projections: first stripe's Q, then K keys 0-255
        # (covers the first two pair fills), then keys 256-511
        emit_qk_proj_chunk(QT_sb, wq_sb, bq_sb, 0, 0)
        emit_k_proj_half(0, 0, 0)

        # ---- the kb-stream ----
        def gmap(g):
            p, kb = divmod(g, NSK)
            mb, qs = divmod(p, NQS)
            return p, mb, qs, kb

        def emit_pair(g, sc_tiles):
            _, mb, qs, kb = gmap(g)
            t = ps.tile([P, 1024], f32, tag="sc", name=f"sc{g}")
            sc_tiles[g] = t
            col = qs * 512
            nc.tensor.matmul(
                t[:, 0:512],
                KT_sb[0:64, mb, kb * P:(kb + 1) * P],
                QT_sb[0:64, mb, col:col + 512],
                start=True, stop=True, tile_position=(0, 0),
            )
            nc.tensor.matmul(
                t[:, 512:1024],
                KT_sb[64:P, mb, kb * P:(kb + 1) * P],
                QT_sb[64:P, mb, col:col + 512],
                start=True, stop=True, tile_position=(64, 0),
            )

        def emit_act(g, sc_tiles, attn_tiles):
            a = attnp.tile([P, 1024], f16, tag="attn", name=f"attn{g}")
            attn_tiles[g] = a
            nc.scalar.activation(a, sc_tiles.pop(g), func=EXP, scale=0.125)

        def emit_attnv(g, attn_tiles, accs):
            p, mb, qs, kb = gmap(g)
            a = attn_tiles.pop(g)
            st = kb == 0
            sp = kb == NSK - 1
            for hi in range(2):
                h = 2 * mb + hi
                nc.tensor.matmul(
                    accs[hi],
                    V_sb[:, kb, h * 65:(h + 1) * 65],
                    a[:, hi * 512:(hi + 1) * 512],
                    start=st, stop=sp,
                )

        # Epilogue is split into 3 stages spread over later periods so the
        # strict-FIFO DVE/Sync queues never head-of-line block on the
        # denominator's DRAM round trips.
        ep_tasks = []  # (due_g, fn)

        def emit_epilogue(p, accs, g0):
            mb, qs = divmod(p, NQS)
            col = qs * 512
            state = {}
            for hi in range(2):
                o_raw = ep.tile([65, 512], f16, tag="oraw", name="o_raw")
                nc.vector.tensor_copy(o_raw, accs[hi])  # frees the acc bank
                state[hi] = o_raw

            def stage2():
                # den broadcast via selector matmul (no DMA round trips),
                # then ~51-ULP reciprocal (den is a sum of positive exps)
                for hi in range(2):
                    o_raw = state[hi]
                    den_bc = pj.tile([64, 512], f32, tag="proj", name="denbc")
                    nc.tensor.matmul(den_bc, sel_sb[0:65, :], o_raw,
                                     start=True, stop=True)
                    rec = ep.tile([64, 512], f32, tag="den", name="rec")
                    nc.vector.reciprocal_approx_fast(rec, den_bc)
                    state[hi] = (o_raw, rec)

            def stage3():
                for hi in range(2):
                    h = 2 * mb + hi
                    o_raw, rec = state[hi]
                    o_fin = ep.tile([64, 512], f16, tag="ofin", name="o_fin")
                    nc.vector.tensor_mul(o_fin, o_raw[0:64, :], rec)
                    nc.vector.tensor_scalar_add(
                        o_fin, o_fin, bv_sb[hi * 64:hi * 64 + 64, mb:mb + 1]
                    )
                    nc.sync.dma_start(
                        out[h * 64:(h + 1) * 64, col:col + 512], o_fin
                    )

            ep_tasks.append((g0 + 1, stage2))
            ep_tasks.append((g0 + 2, stage3))

        def run_ep_tasks(g):
            while ep_tasks and ep_tasks[0][0] <= g:
                ep_tasks.pop(0)[1]()

        sc_tiles = {}
        attn_tiles = {}
        accs = None

        emit_pair(0, sc_tiles)
        emit_pair(1, sc_tiles)
        emit_k_proj_half(0, 0, 1)  # keys 256-511, before fill(2) needs kb2
        for g in range(NG):
            if g % NSK == 0:
                prev_accs = accs
                accs = [accp.tile([65, 512], f32, tag="acc", name=f"acc{g}_{hi}")
                        for hi in range(2)]
            emit_act(g, sc_tiles, attn_tiles)
            if g + 2 < NG:
                emit_pair(g + 2, sc_tiles)
            if g - 1 >= 0:
                emit_attnv(g - 1, attn_tiles,
                           accs if (g - 1) % NSK != NSK - 1 else prev_accs)
                if (g - 1) % NSK == NSK - 1:
                    emit_epilogue((g - 1) // NSK, prev_accs, g)
            run_ep_tasks(g)
            emit_deferred_due(g)
        emit_attnv(NG - 1, attn_tiles, accs)
        emit_epilogue(NPASS - 1, accs, NG)
        for _, fn in ep_tasks:
            fn()
        ep_tasks.clear()

        assert not deferred
        assert not sc_tiles and not attn_tiles

        accp.release()
        pj.release()
        ps.release()
        ep.release()
        attnp.release()
        sb.release()

    nc.finalize()
    return nc


def _get_module():
    global _CACHED
    if _CACHED is None:
        _CACHED = _build_module()
    return _CACHED


def kernel(x, mask, Wq, bq, Wk, bk, Wv, bv):
    from concourse.bass_utils import run_bass_kernel_spmd

    x = np.asarray(x, dtype=np.float32)
    mask = np.asarray(mask, dtype=np.int32)
    Wq = np.asarray(Wq, dtype=np.float32)
    Wk = np.asarray(Wk, dtype=np.float32)
    Wv = np.asarray(Wv, dtype=np.float32)
    bq = np.asarray(bq, dtype=np.float32)
    bk = np.asarray(bk, dtype=np.float32)
    bv = np.asarray(bv, dtype=np.float32)

    nc = _get_module()

    xTs = [np.ascontiguousarray(x[b].T.astype(np.float16)) for b in range(B)]
    maskcs = [np.ascontiguousarray(mask[b].reshape(NSK, P).T) for b in range(B)]

    in_maps = []
    for c in range(N_CORES):
        b, g = divmod(c, 2)
        sl = slice(g * DHC, (g + 1) * DHC)
        biasp = np.empty((P, 9), dtype=np.float32)
        biasp[:, 0:3] = bq[sl].reshape(3, P).T
        biasp[:, 3:6] = bk[sl].reshape(3, P).T
        biasp[:, 6:9] = bv[sl].reshape(3, P).T
        in_maps.append({
            "xT": xTs[b],
            "wq": np.ascontiguousarray(Wq[:, sl].astype(np.float16)),
            "wk": np.ascontiguousarray(Wk[:, sl].astype(np.float16)),
            "wv": np.ascontiguousarray(Wv[:, sl].astype(np.float16)),
            "biasp": biasp,
            "maskc": maskcs[b],
        })

    res = run_bass_kernel_spmd(nc, in_maps, core_ids=list(range(N_CORES)))

    full = np.empty((B, S, D), dtype=np.float32)
    for c in range(N_CORES):
        b, g = divmod(c, 2)
        full[b, :, g * DHC:(g + 1) * DHC] = res.results[c]["out"].T
    return full


# revision 12
# speedup vs baseline: 1.0103x; 1.0103x over previous
"""Multi-headed self-attention Trainium2 kernel (8 NeuronCores), v2.

Problem: B=4, S=2048, D=768, H=12 heads of DH=64; fp32 inputs.
Sharding: core c handles batch b = c//2 and head group g = c%2 (6 heads).

v2 reorganizes the v1 kernel around the ScalarE (ACT) critical path:
the 25.2M-element exp is the steady-state bottleneck (1 elem/lane/cyc
@1.2GHz => 164us floor + 293ns/instr overhead).

  - Score tiles are pair-aligned: one [128,1024] fp32 PSUM tile per key
    block = {head A | head B}, written by a row-tiled concurrent MM pair
    (tile_position (0,0)/(64,0), different PSUM banks). Both halves are
    freed by ONE ACTIVATE, so the next pair's deps clear atomically.
  - Emission is software-pipelined over a continuous kb-stream g=0..191
    (12 stripe-passes x 16 key blocks): per period emit ACT(g),
    pair(g+2), attnV(g-1), then deadline-driven projection chunks.
    No stripe-boundary ACT bubbles.
  - Prologue: 9 coarse DMAs in dependency order (biases+mask, wq,
    xT stripe 0, wk, wv, xT stripes 1-3); PE warmup dummies unthrottle
    the HAM clock gate before the first projection.
  - Epilogue (no DMA round trips; the strict-FIFO DVE/Sync queues must
    never head-of-line block on a multi-us dependency): the acc copy
    writes fp16; a constant selector matmul broadcasts the denominator
    row onto partitions 0-63 in PSUM; reciprocal_approx_fast + mul +
    bias add on DVE; stages are deferred 1-2 periods so each op's
    inputs exist before it reaches its queue head.

Algebra (unchanged from v1): bv is added after normalization (factors
out of the softmax average); the padding mask is folded into V
multiplicatively with a mask-valued denominator column, so attn@V_aug
also produces the softmax denominator (row 64).
"""

import numpy as np

B, S, D, H = 4, 2048, 768, 12
DH = 64          # head dim
HPC = 6          # heads per core
DHC = HPC * DH   # 384 = per-core slice of D
N_CORES = 8
P = 128
KC = D // P      # 6 contraction chunks
NSK = S // P     # 16 key blocks
NQS = S // 512   # 4 query stripes per m-block
NPASS = 3 * NQS  # 12 stripe-passes
NG = NPASS * NSK # 192 kb-stream periods

_CACHED = None


def _build_module():
    import concourse.bacc as bacc
    import concourse.tile as tile
    from concourse import mybir

    f32 = mybir.dt.float32
    f16 = mybir.dt.float16
    i32 = mybir.dt.int32
    EXP = mybir.ActivationFunctionType.Exp

    nc = bacc.Bacc(trn_type="TRN2")

    xT = nc.dram_tensor("xT", [D, S], f16, kind="ExternalInput")
    wq = nc.dram_tensor("wq", [D, DHC], f16, kind="ExternalInput")
    wk = nc.dram_tensor("wk", [D, DHC], f16, kind="ExternalInput")
    wv = nc.dram_tensor("wv", [D, DHC], f16, kind="ExternalInput")
    # bias_pack [128, 9]: cols 0-2 = bq (col=mb), 3-5 = bk, 6-8 = bv
    biasp = nc.dram_tensor("biasp", [P, 9], f32, kind="ExternalInput")
    maskc = nc.dram_tensor("maskc", [P, NSK], i32, kind="ExternalInput")
    out = nc.dram_tensor("out", [DHC, S], f16, kind="ExternalOutput")

    xT4 = xT.rearrange("(c p) s -> p c s", p=P)
    wq3 = wq.rearrange("(c p) n -> p c n", p=P)
    wk3 = wk.rearrange("(c p) n -> p c n", p=P)
    wv3 = wv.rearrange("(c p) n -> p c n", p=P)

    with tile.TileContext(nc) as tc:
        sb = tc.alloc_tile_pool(name="sb", bufs=1)
        attnp = tc.alloc_tile_pool(name="attnp", bufs=10)
        ep = tc.alloc_tile_pool(name="ep", bufs=3)
        # PSUM: 2 score tiles (4 banks) + 2 proj slots (2) + 2 acc (2) = 8
        ps = tc.alloc_tile_pool(name="ps", bufs=2, space="PSUM")
        pj = tc.alloc_tile_pool(name="pj", bufs=2, space="PSUM")
        accp = tc.alloc_tile_pool(name="accp", bufs=2, space="PSUM")

        # ---- PE warmup: dummy matmuls on a scratch tile unthrottle HAM
        warm = sb.tile([P, 512], f16)
        nc.gpsimd.memset(warm, 0)
        # selector for the denominator broadcast: sel.T @ o_raw replicates
        # row 64 (the aug-column denominator) onto partitions 0-63
        sel_sb = sb.tile([P, 64], f16)
        nc.gpsimd.memset(sel_sb, 0)
        nc.gpsimd.memset(sel_sb[64:65, :], 1.0)
        for _ in range(12):
            wps = pj.tile([P, 512], f32, tag="proj", name="warm")
            nc.tensor.matmul(wps, warm[:, 0:128], warm, start=True, stop=True)

        # ---- inputs: coarse DMAs in dependency order ----
        biasp_sb = sb.tile([P, 9], f32)
        nc.sync.dma_start(biasp_sb, biasp[:, :])
        mask_i = sb.tile([P, NSK], i32)
        nc.sync.dma_start(mask_i, maskc[:, :])

        wq_sb = sb.tile([P, KC, DHC], f16)
        nc.sync.dma_start(wq_sb, wq3)

        xT_sb = sb.tile([P, KC, S], f16)
        nc.sync.dma_start(xT_sb[:, :, 0:512], xT4[:, :, 0:512])

        wk_sb = sb.tile([P, KC, DHC], f16)
        nc.sync.dma_start(wk_sb, wk3)

        wv_sb = sb.tile([P, KC, DHC], f16)
        nc.sync.dma_start(wv_sb, wv3)
        for st in range(1, 4):
            nc.sync.dma_start(xT_sb[:, :, st * 512:(st + 1) * 512],
                              xT4[:, :, st * 512:(st + 1) * 512])

        bq_sb = biasp_sb[:, 0:3]
        bk_sb = biasp_sb[:, 3:6]
        bv_sb = biasp_sb[:, 6:9]
        mask_f = sb.tile([P, NSK], f32)
        nc.vector.tensor_copy(mask_f, mask_i)

        # ---- persistent activations ----
        QT_sb = sb.tile([P, 3, S], f16)
        KT_sb = sb.tile([P, 3, S], f16)
        V_sb = sb.tile([P, NSK, HPC * 65], f16)
        V_sb4 = V_sb.rearrange("p n (h e) -> p n h e", e=65)

        def emit_qk_proj_half(dst, w_sb, b_sb, mb, ch, half):
            """256 columns of Q^T or K^T (heads 2mb, 2mb+1): half a chunk,
            so a projection never blocks the score-fill path for >~1us."""
            pps = pj.tile([P, 256], f32, tag="proj", name="pps")
            col = ch * 512 + half * 256
            for c in range(KC):
                nc.tensor.matmul(
                    pps,
                    w_sb[:, c, mb * P:(mb + 1) * P],
                    xT_sb[:, c, col:col + 256],
                    start=(c == 0), stop=(c == KC - 1),
                )
            nc.vector.tensor_scalar(
                dst[:, mb, col:col + 256], pps,
                b_sb[:, mb:mb + 1], None, mybir.AluOpType.add,
            )

        def emit_v_proj_chunk(sk):
            vps = pj.tile([P, DHC], f32, tag="proj", name="vps")
            for c in range(KC):
                nc.tensor.matmul(
                    vps,
                    xT_sb[:, c, sk * P:(sk + 1) * P],
                    wv_sb[:, c, :],
                    start=(c == 0), stop=(c == KC - 1),
                )
            # evac with the multiplicative mask; fp32 -> fp16
            nc.vector.tensor_scalar(
                V_sb4[:, sk, :, 0:64],
                vps.rearrange("p (h e) -> p h e", e=64),
                mask_f[:, sk:sk + 1], None, mybir.AluOpType.mult,
            )
            nc.vector.tensor_copy(
                V_sb4[:, sk, :, 64],
                mask_f[:, sk:sk + 1].to_broadcast([P, HPC]),
            )

        # deferred projection chunks with deadlines in kb-stream units
        deferred = []
        for sk in range(NSK):
            deferred.append((sk + 2, "v", sk))
        for mb in range(3):
            for ch in range(NQS):
                if not (mb == 0 and ch == 0):
                    d = (mb * NQS + ch) * NSK
                    deferred.append((d - 3, "q", mb, ch, 0))
                    deferred.append((d - 2, "q", mb, ch, 1))
                    deferred.append((mb * 4 * NSK + ch * 4 - 2, "k", mb, ch, 0))
                    deferred.append((mb * 4 * NSK + ch * 4, "k", mb, ch, 1))
        deferred.sort(key=lambda t: t[0])

        def emit_deferred_due(g):
            while deferred and deferred[0][0] <= g + 2:
                item = deferred.pop(0)
                if item[1] == "v":
                    emit_v_proj_chunk(item[2])
                elif item[1] == "q":
                    emit_qk_proj_half(QT_sb, wq_sb, bq_sb,
                                      item[2], item[3], item[4])
                else:
                    emit_qk_proj_half(KT_sb, wk_sb, bk_sb,
                                      item[2], item[3], item[4])

        # prologue projections: first stripe's Q, then K keys 0-255
        # (covers the first two pair fills), then keys 256-511
        emit_qk_proj_half(QT_sb, wq_sb, bq_sb, 0, 0, 0)
        emit_qk_proj_half(QT_sb, wq_sb, bq_sb, 0, 0, 1)
        emit_qk_proj_half(KT_sb, wk_sb, bk_sb, 0, 0, 0)

        # ---- the kb-stream ----
        def gmap(g):
            p, kb = divmod(g, NSK)
            mb, qs = divmod(p, NQS)
            return p, mb, qs, kb

        def emit_pair(g, sc_tiles):
            _, mb, qs, kb = gmap(g)
            t = ps.tile([P, 1024], f32, tag="sc", name=f"sc{g}")
            sc_tiles[g] = t
            col = qs * 512
            nc.tensor.matmul(
                t[:, 0:512],
                KT_sb[0:64, mb, kb * P:(kb + 1) * P],
                QT_sb[0:64, mb, col:col + 512],
                start=True, stop=True, tile_position=(0, 0),
            )
            nc.tensor.matmul(
                t[:, 512:1024],
                KT_sb[64:P, mb, kb * P:(kb + 1) * P],
                QT_sb[64:P, mb, col:col + 512],
                start=True, stop=True, tile_position=(64, 0),
            )

        def emit_act(g, sc_tiles, attn_tiles):
            a = attnp.tile([P, 1024], f16, tag="attn", name=f"attn{g}")
            attn_tiles[g] = a
            nc.scalar.activation(a, sc_tiles.pop(g), func=EXP, scale=0.125)

        def emit_attnv(g, attn_tiles, accs):
            p, mb, qs, kb = gmap(g)
            a = attn_tiles.pop(g)
            st = kb == 0
            sp = kb == NSK - 1
            for hi in range(2):
                h = 2 * mb + hi
                nc.tensor.matmul(
                    accs[hi],
                    V_sb[:, kb, h * 65:(h + 1) * 65],
                    a[:, hi * 512:(hi + 1) * 512],
                    start=st, stop=sp,
                )

        # Epilogue is split into 3 stages spread over later periods so the
        # strict-FIFO DVE/Sync queues never head-of-line block on the
        # denominator's DRAM round trips.
        ep_tasks = []  # (due_g, fn)

        def emit_epilogue(p, accs, g0):
            mb, qs = divmod(p, NQS)
            col = qs * 512
            state = {}
            for hi in range(2):
                o_raw = ep.tile([65, 512], f16, tag="oraw", name="o_raw")
                nc.vector.tensor_copy(o_raw, accs[hi])  # frees the acc bank
                state[hi] = o_raw

            def stage2():
                # den broadcast via selector matmul (no DMA round trips),
                # then ~51-ULP reciprocal (den is a sum of positive exps)
                for hi in range(2):
                    o_raw = state[hi]
                    den_bc = pj.tile([64, 512], f32, tag="proj", name="denbc")
                    nc.tensor.matmul(den_bc, sel_sb[0:65, :], o_raw,
                                     start=True, stop=True)
                    rec = ep.tile([64, 512], f32, tag="den", name="rec")
                    nc.vector.reciprocal_approx_fast(rec, den_bc)
                    state[hi] = (o_raw, rec)

            def stage3():
                for hi in range(2):
                    h = 2 * mb + hi
                    o_raw, rec = state[hi]
                    o_fin = ep.tile([64, 512], f16, tag="ofin", name="o_fin")
                    nc.vector.tensor_mul(o_fin, o_raw[0:64, :], rec)
                    nc.vector.tensor_scalar_add(
                        o_fin, o_fin, bv_sb[hi * 64:hi * 64 + 64, mb:mb + 1]
                    )
                    nc.sync.dma_start(
                        out[h * 64:(h + 1) * 64, col:col + 512], o_fin
                    )

            ep_tasks.append((g0 + 1, stage2))
            ep_tasks.append((g0 + 2, stage3))

        def run_ep_tasks(g):
            while ep_tasks and ep_tasks[0][0] <= g:
                ep_tasks.pop(0)[1]()

        sc_tiles = {}
        attn_tiles = {}
        accs = None

        emit_pair(0, sc_tiles)
        emit_pair(1, sc_tiles)
        # keys 256-511, before fill(2) needs kb2
        emit_qk_proj_half(KT_sb, wk_sb, bk_sb, 0, 0, 1)
        for g in range(NG):
            if g % NSK == 0:
                prev_accs = accs
                accs = [accp.tile([65, 512], f32, tag="acc", name=f"acc{g}_{hi}")
                        for hi in range(2)]
            emit_act(g, sc_tiles, attn_tiles)
            if g + 2 < NG:
                emit_pair(g + 2, sc_tiles)
            if g - 1 >= 0:
                emit_attnv(g - 1, attn_tiles,
                           accs if (g - 1) % NSK != NSK - 1 else prev_accs)
                if (g - 1) % NSK == NSK - 1:
                    emit_epilogue((g - 1) // NSK, prev_accs, g)
            run_ep_tasks(g)
            emit_deferred_due(g)
        emit_attnv(NG - 1, attn_tiles, accs)
        emit_epilogue(NPASS - 1, accs, NG)
        for _, fn in ep_tasks:
            fn()
        ep_tasks.clear()

        assert not deferred
        assert not sc_tiles and not attn_tiles

        accp.release()
        pj.release()
        ps.release()
        ep.release()
        attnp.release()
        sb.release()

    nc.finalize()
    return nc


def _get_module():
    global _CACHED
    if _CACHED is None:
        _CACHED = _build_module()
    return _CACHED


def kernel(x, mask, Wq, bq, Wk, bk, Wv, bv):
    from concourse.bass_utils import run_bass_kernel_spmd

    x = np.asarray(x, dtype=np.float32)
    mask = np.asarray(mask, dtype=np.int32)
    Wq = np.asarray(Wq, dtype=np.float32)
    Wk = np.asarray(Wk, dtype=np.float32)
    Wv = np.asarray(Wv, dtype=np.float32)
    bq = np.asarray(bq, dtype=np.float32)
    bk = np.asarray(bk, dtype=np.float32)
    bv = np.asarray(bv, dtype=np.float32)

    nc = _get_module()

    xTs = [np.ascontiguousarray(x[b].T.astype(np.float16)) for b in range(B)]
    maskcs = [np.ascontiguousarray(mask[b].reshape(NSK, P).T) for b in range(B)]

    in_maps = []
    for c in range(N_CORES):
        b, g = divmod(c, 2)
        sl = slice(g * DHC, (g + 1) * DHC)
        biasp = np.empty((P, 9), dtype=np.float32)
        biasp[:, 0:3] = bq[sl].reshape(3, P).T
        biasp[:, 3:6] = bk[sl].reshape(3, P).T
        biasp[:, 6:9] = bv[sl].reshape(3, P).T
        in_maps.append({
            "xT": xTs[b],
            "wq": np.ascontiguousarray(Wq[:, sl].astype(np.float16)),
            "wk": np.ascontiguousarray(Wk[:, sl].astype(np.float16)),
            "wv": np.ascontiguousarray(Wv[:, sl].astype(np.float16)),
            "biasp": biasp,
            "maskc": maskcs[b],
        })

    res = run_bass_kernel_spmd(nc, in_maps, core_ids=list(range(N_CORES)))

    full = np.empty((B, S, D), dtype=np.float32)
    for c in range(N_CORES):
        b, g = divmod(c, 2)
        full[b, :, g * DHC:(g + 1) * DHC] = res.results[c]["out"].T
    return full
